# revision 6
# baseline (speedup 1.0000x reference)
"""Trainium2 Bass kernel for BandSplitModule (masked LN per band + weight-normed Linear).

Strategy:
  - Data-parallel over T (2048 = 8 cores x 256). No collectives.
  - Host folds weight-norm + LN affine into a single per-band weight matrix
    W2[n] = (g * v / ||v||) * (gamma * mask) with a bias row
    bias2[n] = W @ (beta * mask) + bias, prepended as contraction row 0
    (the device appends a ones column to xhat so the matmul adds the bias).
  - Features permuted from (c, k, reim) to (k, c, reim) order so each band's
    features are one contiguous slice of a [t=128, (F+64)*4] SBUF slab.
  - Runtime band_start/band_width are baked into the compiled program
    (compilation happens inside kernel(); results cached per band structure).
  - Device per band-tile: bn_stats/bn_aggr -> rsqrt -> tensor_scalar xhat,
    TensorE transpose -> matmul (k = 4w+1 chunks) -> z[E, T] psum -> out.
"""
import os
import numpy as np

B, C, F, T, E = 4, 2, 1025, 2048, 128
MAX_BW = 65
NB = 37
EPS = 1e-5
NCORES = 8
TLOC = T // NCORES  # 256
FPAD = F + MAX_BW - 1  # 1089
D = C * MAX_BW * 2  # 260

LAST_EXEC_NS = None

_PLAN_CACHE = {}


def _ensure_trace_hook():
    """Install the antenv.axon_hooks NTFF-profile shim (missing on this image)
    so run_bass_kernel_spmd(trace=True) can capture HW exec time. Fully
    optional — any failure leaves the plain execution path untouched."""
    try:
        import sys, types

        if "antenv.axon_hooks" not in sys.modules:
            mod = types.ModuleType("antenv.axon_hooks")
            _h = {"hook": None}
            mod.set_axon_ntff_profile_hook = lambda h: _h.__setitem__("hook", h)
            mod.get_axon_ntff_profile_hook = lambda: _h["hook"]
            sys.modules["antenv.axon_hooks"] = mod
            try:
                import antenv

                antenv.axon_hooks = mod
            except Exception:
                pass
            try:
                from trn_agent_boot.trn_boot import _ntff_profile_via_ctypes

                hook = _ntff_profile_via_ctypes("/opt/axon/libaxon_pjrt.so")
                if hook is not None:
                    mod.set_axon_ntff_profile_hook(hook)
            except Exception:
                pass
        import concourse.bass_utils as bu

        if not getattr(bu, "_offline_upload_patch", False):
            bu.upload_artifacts = lambda tmpdir: tmpdir
            bu._offline_upload_patch = True
    except Exception:
        pass


def _feature_perm():
    # new index (k,c,r) -> reference index (c,k,r)
    kk, cc, rr = np.meshgrid(
        np.arange(MAX_BW), np.arange(C), np.arange(2), indexing="ij"
    )
    new_i = (kk * 4 + cc * 2 + rr).reshape(-1)
    src_i = (cc * (MAX_BW * 2) + kk * 2 + rr).reshape(-1)
    perm = np.empty(D, np.int64)
    perm[new_i] = src_i
    return perm


def _fold_weights(ln_gamma, ln_beta, v, g, bias, widths):
    karr = np.arange(MAX_BW)
    bw_mask = karr[None, :] < widths[:, None]
    fm = (
        np.broadcast_to(bw_mask[:, None, :, None], (NB, C, MAX_BW, 2))
        .reshape(NB, D)
        .astype(np.float32)
    )
    vnorm = np.sqrt((v * v).sum(-1, keepdims=True))
    W = g[..., None] * v / vnorm
    W2 = W * (ln_gamma * fm)[:, None, :]
    bias2 = np.einsum("ned,nd->ne", W, ln_beta * fm) + bias
    W2p = W2[:, :, _feature_perm()]  # [NB, E, D] in (k,c,r) order
    return W2p, bias2


def _pack_weights(W2p, bias2, widths):
    """Pack per-band [k_n = 4w+1, E] (row0 = bias) into SBUF-layout chunks of 128."""
    kns = (4 * widths + 1).astype(np.int64)
    nchunks = np.maximum(1, (kns + 127) // 128).astype(np.int64)
    tot_chunks = int(nchunks.sum())
    Wt = np.zeros((128, tot_chunks * 128), np.float32)
    chunk_base = np.zeros(NB, np.int64)
    cb = 0
    for n in range(NB):
        chunk_base[n] = cb
        kn = int(kns[n])
        w4 = kn - 1
        col = np.zeros((kn, E), np.float32)
        col[0] = bias2[n]
        if w4 > 0:
            col[1:] = W2p[n, :, :w4].T
        for j in range(int(nchunks[n])):
            cs = min(128, kn - j * 128)
            Wt[:cs, (cb + j) * 128 : (cb + j) * 128 + E] = col[j * 128 : j * 128 + cs]
        cb += int(nchunks[n])
    return Wt, kns, nchunks, chunk_base, tot_chunks


def _prep_x(x):
    """x [B,C,F,T,2] f32 -> x4 [NCORES, B, TLOC, FPAD*4] with (k,c,r) features, padded."""
    xr = np.transpose(x, (0, 3, 2, 1, 4)).reshape(B, T, F, 4)  # [B,T,F,(c,r)]
    x4 = np.empty((B, T, FPAD, 4), np.float32)
    x4[:, :, :F, :] = xr
    x4[:, :, F:, :] = xr[:, :, F - 1 : F, :]
    x4 = x4.reshape(B, NCORES, TLOC, FPAD * 4)
    x4 = np.ascontiguousarray(np.transpose(x4, (1, 0, 2, 3)))
    return x4  # [NCORES, B, TLOC, FPAD*4]


def _build_program(kns, nchunks, chunk_base, tot_chunks, starts):
    import concourse.bass as bass
    import concourse.bacc as bacc
    import concourse.tile as tile
    from concourse import mybir
    from concourse.masks import make_identity
    from contextlib import ExitStack

    f32 = mybir.dt.float32
    nc = bacc.Bacc()
    x_ext = nc.declare_dram_parameter("x4", [B, TLOC, FPAD * 4], f32, isOutput=False)
    wt_ext = nc.declare_dram_parameter(
        "wt", [128, tot_chunks * 128], f32, isOutput=False
    )
    z_ext = nc.declare_dram_parameter("out", [NB, B, E, TLOC], f32, isOutput=True)

    with ExitStack() as ctx:
        tc = ctx.enter_context(tile.TileContext(nc))
        consts = ctx.enter_context(tc.tile_pool(name="consts", bufs=1))
        slabs = ctx.enter_context(tc.tile_pool(name="slabs", bufs=1))
        stats = ctx.enter_context(tc.tile_pool(name="stats", bufs=8))
        xh_pool = ctx.enter_context(tc.tile_pool(name="xh", bufs=4))
        xt_pool = ctx.enter_context(tc.tile_pool(name="xt", bufs=6))
        zs_pool = ctx.enter_context(tc.tile_pool(name="zs", bufs=4))
        tp_psum = ctx.enter_context(tc.tile_pool(name="tp", bufs=4, space="PSUM"))
        z_psum = ctx.enter_context(tc.tile_pool(name="zp", bufs=3, space="PSUM"))

        ident = consts.tile([128, 128], f32)
        make_identity(nc, ident)
        eps_t = consts.tile([128, 1], f32)
        nc.vector.memset(eps_t, EPS)
        wt_sb = consts.tile([128, tot_chunks * 128], f32)
        nc.sync.dma_start(out=wt_sb, in_=wt_ext[:, :])

        slab_tiles = {}
        for b in range(B):
            for t0 in range(TLOC // 128):
                st = slabs.tile([128, FPAD * 4], f32, tag=f"slab_{b}_{t0}")
                nc.sync.dma_start(
                    out=st, in_=x_ext[b, t0 * 128 : (t0 + 1) * 128, :]
                )
                slab_tiles[(b, t0)] = st

        for n in range(NB):
            kn = int(kns[n])
            w4 = kn - 1
            s4 = 4 * int(starts[n])
            nch = int(nchunks[n])
            cb = int(chunk_base[n])
            for b in range(B):
                for t0 in range(TLOC // 128):
                    sl = slab_tiles[(b, t0)]
                    xh_t = xh_pool.tile([128, 264], f32)
                    nc.gpsimd.memset(xh_t[:, 0:1], 1.0)
                    if w4 > 0:
                        xsl = sl[:, s4 : s4 + w4]
                        stt = stats.tile([128, 6], f32)
                        nc.vector.bn_stats(out=stt, in_=xsl)
                        mv = stats.tile([128, 2], f32)
                        nc.vector.bn_aggr(out=mv, in_=stt)
                        rs = stats.tile([128, 1], f32)
                        nc.scalar.activation(
                            out=rs,
                            in_=mv[:, 1:2],
                            func=mybir.ActivationFunctionType.Sqrt,
                            bias=eps_t,
                            scale=1.0,
                        )
                        nc.vector.reciprocal(out=rs, in_=rs)
                        nc.vector.tensor_scalar(
                            out=xh_t[:, 1:kn],
                            in0=xsl,
                            scalar1=mv[:, 0:1],
                            scalar2=rs,
                            op0=mybir.AluOpType.subtract,
                            op1=mybir.AluOpType.mult,
                        )
                    zp = z_psum.tile([128, 128], f32)
                    for j in range(nch):
                        cs = min(128, kn - j * 128)
                        tp = tp_psum.tile([128, 128], f32)
                        nc.tensor.transpose(
                            out=tp[:cs, :],
                            in_=xh_t[:, j * 128 : j * 128 + cs],
                            identity=ident,
                        )
                        xt = xt_pool.tile([128, 128], f32)
                        nc.any.tensor_copy(out=xt[:cs, :], in_=tp[:cs, :])
                        nc.tensor.matmul(
                            zp,
                            lhsT=wt_sb[:cs, (cb + j) * 128 : (cb + j) * 128 + E],
                            rhs=xt[:cs, :],
                            start=(j == 0),
                            stop=(j == nch - 1),
                        )
                    zs = zs_pool.tile([128, 128], f32)
                    nc.any.tensor_copy(out=zs, in_=zp)
                    nc.sync.dma_start(
                        out=z_ext[n, b, :, t0 * 128 : (t0 + 1) * 128], in_=zs
                    )
    nc.compile()
    return nc


def kernel(x, ln_gamma, ln_beta, v, g, bias, band_start, band_width):
    global LAST_EXEC_NS
    _ensure_trace_hook()
    from concourse.bass_utils import run_bass_kernel_spmd

    x = np.asarray(x, np.float32)
    ln_gamma = np.asarray(ln_gamma, np.float32)
    ln_beta = np.asarray(ln_beta, np.float32)
    v = np.asarray(v, np.float32)
    g = np.asarray(g, np.float32)
    bias = np.asarray(bias, np.float32)
    starts = np.asarray(band_start).astype(np.int64)
    widths = np.asarray(band_width).astype(np.int64)

    W2p, bias2 = _fold_weights(ln_gamma, ln_beta, v, g, bias, widths)
    Wt, kns, nchunks, chunk_base, tot_chunks = _pack_weights(W2p, bias2, widths)
    x4 = _prep_x(x)

    key = (tuple(starts.tolist()), tuple(widths.tolist()))
    if key not in _PLAN_CACHE:
        _PLAN_CACHE[key] = _build_program(
            kns, nchunks, chunk_base, tot_chunks, starts
        )
    nc = _PLAN_CACHE[key]

    in_maps = [{"x4": x4[i], "wt": Wt} for i in range(NCORES)]
    res = run_bass_kernel_spmd(nc, in_maps, core_ids=list(range(NCORES)))
    LAST_EXEC_NS = res.exec_time_ns

    zarr = np.stack([r["out"] for r in res.results])  # [8, NB, B, E, TLOC]
    z = np.transpose(zarr, (2, 1, 0, 4, 3)).reshape(B, NB, T, E)
    return np.ascontiguousarray(z)


# revision 11
# speedup vs baseline: 1.5695x; 1.5695x over previous
"""Trainium2 Bass kernel for BandSplitModule (masked LN per band + weight-normed Linear).

Strategy:
  - Data-parallel over T (2048 = 8 cores x 256). No collectives.
  - Host folds weight-norm + LN affine into a single per-band weight matrix
    W2[n] = (g * v / ||v||) * (gamma * mask) with a bias row
    bias2[n] = W @ (beta * mask) + bias, prepended as contraction row 0
    (the device appends a ones column to xhat so the matmul adds the bias).
  - Features permuted from (c, k, reim) to (k, c, reim) order so each band's
    features are one contiguous slice of a [t=128, (F+64)*4] SBUF slab.
  - Runtime band_start/band_width are baked into the compiled program
    (compilation happens inside kernel(); results cached per band structure).
  - Device per band-tile: bn_stats/bn_aggr -> rsqrt -> tensor_scalar xhat,
    TensorE transpose -> matmul (k = 4w+1 chunks) -> z[E, T] psum -> out.
"""
import os
import numpy as np

B, C, F, T, E = 4, 2, 1025, 2048, 128
MAX_BW = 65
NB = 37
EPS = 1e-5
NCORES = 8
TLOC = T // NCORES  # 256
FPAD = F + MAX_BW - 1  # 1089
D = C * MAX_BW * 2  # 260

LAST_EXEC_NS = None

_PLAN_CACHE = {}


def _ensure_trace_hook():
    """Install the antenv.axon_hooks NTFF-profile shim (missing on this image)
    so run_bass_kernel_spmd(trace=True) can capture HW exec time. Fully
    optional — any failure leaves the plain execution path untouched."""
    try:
        import sys, types

        if "antenv.axon_hooks" not in sys.modules:
            mod = types.ModuleType("antenv.axon_hooks")
            _h = {"hook": None}
            mod.set_axon_ntff_profile_hook = lambda h: _h.__setitem__("hook", h)
            mod.get_axon_ntff_profile_hook = lambda: _h["hook"]
            sys.modules["antenv.axon_hooks"] = mod
            try:
                import antenv

                antenv.axon_hooks = mod
            except Exception:
                pass
            try:
                from trn_agent_boot.trn_boot import _ntff_profile_via_ctypes

                hook = _ntff_profile_via_ctypes("/opt/axon/libaxon_pjrt.so")
                if hook is not None:
                    mod.set_axon_ntff_profile_hook(hook)
            except Exception:
                pass
        import concourse.bass_utils as bu

        if not getattr(bu, "_offline_upload_patch", False):
            bu.upload_artifacts = lambda tmpdir: tmpdir
            bu._offline_upload_patch = True
    except Exception:
        pass


def _feature_perm():
    # new index (k,c,r) -> reference index (c,k,r)
    kk, cc, rr = np.meshgrid(
        np.arange(MAX_BW), np.arange(C), np.arange(2), indexing="ij"
    )
    new_i = (kk * 4 + cc * 2 + rr).reshape(-1)
    src_i = (cc * (MAX_BW * 2) + kk * 2 + rr).reshape(-1)
    perm = np.empty(D, np.int64)
    perm[new_i] = src_i
    return perm


def _fold_weights(ln_gamma, ln_beta, v, g, bias, widths):
    karr = np.arange(MAX_BW)
    bw_mask = karr[None, :] < widths[:, None]
    fm = (
        np.broadcast_to(bw_mask[:, None, :, None], (NB, C, MAX_BW, 2))
        .reshape(NB, D)
        .astype(np.float32)
    )
    vnorm = np.sqrt((v * v).sum(-1, keepdims=True))
    W = g[..., None] * v / vnorm
    W2 = W * (ln_gamma * fm)[:, None, :]
    bias2 = np.einsum("ned,nd->ne", W, ln_beta * fm) + bias
    W2p = W2[:, :, _feature_perm()]  # [NB, E, D] in (k,c,r) order
    return W2p, bias2


def _pack_weights(W2p, widths):
    """Pack per-band [k_n = 4w, E] weight rows into SBUF-layout chunks of 128."""
    kns = np.maximum(4 * widths, 4).astype(np.int64)
    nchunks = np.maximum(1, (kns + 127) // 128).astype(np.int64)
    tot_chunks = int(nchunks.sum())
    Wt = np.zeros((128, tot_chunks * 128), np.float32)
    chunk_base = np.zeros(NB, np.int64)
    cb = 0
    for n in range(NB):
        chunk_base[n] = cb
        kn = int(kns[n])
        w4 = 4 * int(widths[n])
        col = np.zeros((kn, E), np.float32)
        if w4 > 0:
            col[:w4] = W2p[n, :, :w4].T
        for j in range(int(nchunks[n])):
            cs = min(128, kn - j * 128)
            Wt[:cs, (cb + j) * 128 : (cb + j) * 128 + E] = col[j * 128 : j * 128 + cs]
        cb += int(nchunks[n])
    return Wt, kns, nchunks, chunk_base, tot_chunks


def _prep_x(x):
    """x [B,C,F,T,2] f32 -> x4 [NCORES, B, TLOC, FPAD*4] with (k,c,r) features, padded."""
    xr = np.transpose(x, (0, 3, 2, 1, 4)).reshape(B, T, F, 4)  # [B,T,F,(c,r)]
    x4 = np.empty((B, T, FPAD, 4), np.float32)
    x4[:, :, :F, :] = xr
    x4[:, :, F:, :] = xr[:, :, F - 1 : F, :]
    x4 = x4.reshape(B, NCORES, TLOC, FPAD * 4)
    x4 = np.ascontiguousarray(np.transpose(x4, (1, 0, 2, 3)))
    return x4  # [NCORES, B, TLOC, FPAD*4]


def _build_program(kns, nchunks, chunk_base, tot_chunks, starts):
    import concourse.bass as bass
    import concourse.bacc as bacc
    import concourse.tile as tile
    from concourse import mybir
    from concourse.masks import make_identity
    from contextlib import ExitStack

    f32 = mybir.dt.float32
    bf16 = mybir.dt.bfloat16
    nc = bacc.Bacc()
    x_ext = nc.declare_dram_parameter("x4", [B, TLOC, FPAD * 4], bf16, isOutput=False)
    wt_ext = nc.declare_dram_parameter(
        "wt", [128, tot_chunks * 128], bf16, isOutput=False
    )
    bias_ext = nc.declare_dram_parameter("bias2", [E, NB], f32, isOutput=False)
    z_ext = nc.declare_dram_parameter("out", [NB, B, E, TLOC], bf16, isOutput=True)

    with ExitStack() as ctx:
        tc = ctx.enter_context(tile.TileContext(nc))
        consts = ctx.enter_context(tc.tile_pool(name="consts", bufs=1))
        slabs = ctx.enter_context(tc.tile_pool(name="slabs", bufs=1))
        stats = ctx.enter_context(tc.tile_pool(name="stats", bufs=8))
        xh_pool = ctx.enter_context(tc.tile_pool(name="xh", bufs=4))
        xt_pool = ctx.enter_context(tc.tile_pool(name="xt", bufs=6))
        zs_pool = ctx.enter_context(tc.tile_pool(name="zs", bufs=4))
        tp_psum = ctx.enter_context(tc.tile_pool(name="tp", bufs=4, space="PSUM"))
        z_psum = ctx.enter_context(tc.tile_pool(name="zp", bufs=3, space="PSUM"))

        ident = consts.tile([128, 128], bf16)
        make_identity(nc, ident)
        eps_t = consts.tile([128, 1], f32)
        nc.vector.memset(eps_t, EPS)
        wt_sb = consts.tile([128, tot_chunks * 128], bf16)
        nc.sync.dma_start(out=wt_sb, in_=wt_ext[:, :])
        bias_sb = consts.tile([E, NB], f32)
        nc.sync.dma_start(out=bias_sb, in_=bias_ext[:, :])

        slab_tiles = {}
        for b in range(B):
            for t0 in range(TLOC // 128):
                st = slabs.tile([128, FPAD * 4], bf16, tag=f"slab_{b}_{t0}")
                nc.sync.dma_start(
                    out=st, in_=x_ext[b, t0 * 128 : (t0 + 1) * 128, :]
                )
                slab_tiles[(b, t0)] = st

        for n in range(NB):
            kn = int(kns[n])
            s4 = 4 * int(starts[n])
            nch = int(nchunks[n])
            cb = int(chunk_base[n])
            for b in range(B):
                for t0 in range(TLOC // 128):
                    sl = slab_tiles[(b, t0)]
                    xsl = sl[:, s4 : s4 + kn]
                    stt = stats.tile([128, 6], f32)
                    nc.vector.bn_stats(out=stt, in_=xsl)
                    mv = stats.tile([128, 2], f32)
                    nc.vector.bn_aggr(out=mv, in_=stt)
                    rs = stats.tile([128, 1], f32)
                    nc.scalar.activation(
                        out=rs,
                        in_=mv[:, 1:2],
                        func=mybir.ActivationFunctionType.Sqrt,
                        bias=eps_t,
                        scale=1.0,
                    )
                    nc.vector.reciprocal(out=rs, in_=rs)
                    xh_t = xh_pool.tile([128, 260], bf16)
                    nc.vector.tensor_scalar(
                        out=xh_t[:, :kn],
                        in0=xsl,
                        scalar1=mv[:, 0:1],
                        scalar2=rs,
                        op0=mybir.AluOpType.subtract,
                        op1=mybir.AluOpType.mult,
                    )
                    zp = z_psum.tile([128, 128], f32)
                    for j in range(nch):
                        cs = min(128, kn - j * 128)
                        tp = tp_psum.tile([128, 128], bf16)
                        nc.tensor.transpose(
                            out=tp[:cs, :],
                            in_=xh_t[:, j * 128 : j * 128 + cs],
                            identity=ident,
                        )
                        xt = xt_pool.tile([128, 128], bf16)
                        nc.any.tensor_copy(out=xt[:cs, :], in_=tp[:cs, :])
                        nc.tensor.matmul(
                            zp,
                            lhsT=wt_sb[:cs, (cb + j) * 128 : (cb + j) * 128 + E],
                            rhs=xt[:cs, :],
                            start=(j == 0),
                            stop=(j == nch - 1),
                        )
                    zs = zs_pool.tile([128, 128], bf16)
                    nc.scalar.activation(
                        out=zs,
                        in_=zp,
                        func=mybir.ActivationFunctionType.Identity,
                        bias=bias_sb[:, n : n + 1],
                        scale=1.0,
                    )
                    nc.sync.dma_start(
                        out=z_ext[n, b, :, t0 * 128 : (t0 + 1) * 128], in_=zs
                    )
    nc.compile()
    return nc


def kernel(x, ln_gamma, ln_beta, v, g, bias, band_start, band_width):
    global LAST_EXEC_NS
    _ensure_trace_hook()
    from concourse.bass_utils import run_bass_kernel_spmd

    x = np.asarray(x, np.float32)
    ln_gamma = np.asarray(ln_gamma, np.float32)
    ln_beta = np.asarray(ln_beta, np.float32)
    v = np.asarray(v, np.float32)
    g = np.asarray(g, np.float32)
    bias = np.asarray(bias, np.float32)
    starts = np.asarray(band_start).astype(np.int64)
    widths = np.asarray(band_width).astype(np.int64)

    import ml_dtypes

    W2p, bias2 = _fold_weights(ln_gamma, ln_beta, v, g, bias, widths)
    Wt, kns, nchunks, chunk_base, tot_chunks = _pack_weights(W2p, widths)
    x4 = _prep_x(x)

    bf = ml_dtypes.bfloat16
    x4b = x4.astype(bf)
    Wtb = Wt.astype(bf)
    bias2t = np.ascontiguousarray(bias2.T)  # [E, NB] f32

    key = (tuple(starts.tolist()), tuple(widths.tolist()))
    if key not in _PLAN_CACHE:
        _PLAN_CACHE[key] = _build_program(
            kns, nchunks, chunk_base, tot_chunks, starts
        )
    nc = _PLAN_CACHE[key]

    in_maps = [
        {"x4": x4b[i], "wt": Wtb, "bias2": bias2t} for i in range(NCORES)
    ]
    res = run_bass_kernel_spmd(nc, in_maps, core_ids=list(range(NCORES)))
    LAST_EXEC_NS = res.exec_time_ns

    zarr = np.stack([np.asarray(r["out"]) for r in res.results]).astype(
        np.float32
    )  # [8, NB, B, E, TLOC]
    z = np.transpose(zarr, (2, 1, 0, 4, 3)).reshape(B, NB, T, E)
    return np.ascontiguousarray(z)


# revision 15
# speedup vs baseline: 2.0759x; 1.3227x over previous
"""Trainium2 Bass kernel for BandSplitModule (masked LN per band + weight-normed Linear).

Strategy:
  - Data-parallel over T (2048 = 8 cores x 256). No collectives.
  - Host folds weight-norm + LN affine into a single per-band weight matrix
    W2[n] = (g * v / ||v||) * (gamma * mask) with a bias row
    bias2[n] = W @ (beta * mask) + bias, prepended as contraction row 0
    (the device appends a ones column to xhat so the matmul adds the bias).
  - Features permuted from (c, k, reim) to (k, c, reim) order so each band's
    features are one contiguous slice of a [t=128, (F+64)*4] SBUF slab.
  - Runtime band_start/band_width are baked into the compiled program
    (compilation happens inside kernel(); results cached per band structure).
  - Device per band-tile: bn_stats/bn_aggr -> rsqrt -> tensor_scalar xhat,
    TensorE transpose -> matmul (k = 4w+1 chunks) -> z[E, T] psum -> out.
"""
import os
import numpy as np

B, C, F, T, E = 4, 2, 1025, 2048, 128
MAX_BW = 65
NB = 37
EPS = 1e-5
NCORES = 8
TLOC = T // NCORES  # 256
FPAD = F + MAX_BW - 1  # 1089
D = C * MAX_BW * 2  # 260

LAST_EXEC_NS = None

_PLAN_CACHE = {}


def _ensure_trace_hook():
    """Install the antenv.axon_hooks NTFF-profile shim (missing on this image)
    so run_bass_kernel_spmd(trace=True) can capture HW exec time. Fully
    optional — any failure leaves the plain execution path untouched."""
    try:
        import sys, types

        if "antenv.axon_hooks" not in sys.modules:
            mod = types.ModuleType("antenv.axon_hooks")
            _h = {"hook": None}
            mod.set_axon_ntff_profile_hook = lambda h: _h.__setitem__("hook", h)
            mod.get_axon_ntff_profile_hook = lambda: _h["hook"]
            sys.modules["antenv.axon_hooks"] = mod
            try:
                import antenv

                antenv.axon_hooks = mod
            except Exception:
                pass
            try:
                from trn_agent_boot.trn_boot import _ntff_profile_via_ctypes

                hook = _ntff_profile_via_ctypes("/opt/axon/libaxon_pjrt.so")
                if hook is not None:
                    mod.set_axon_ntff_profile_hook(hook)
            except Exception:
                pass
        import concourse.bass_utils as bu

        if not getattr(bu, "_offline_upload_patch", False):
            bu.upload_artifacts = lambda tmpdir: tmpdir
            bu._offline_upload_patch = True
    except Exception:
        pass


def _feature_perm():
    # new index (k,c,r) -> reference index (c,k,r)
    kk, cc, rr = np.meshgrid(
        np.arange(MAX_BW), np.arange(C), np.arange(2), indexing="ij"
    )
    new_i = (kk * 4 + cc * 2 + rr).reshape(-1)
    src_i = (cc * (MAX_BW * 2) + kk * 2 + rr).reshape(-1)
    perm = np.empty(D, np.int64)
    perm[new_i] = src_i
    return perm


def _fold_weights(ln_gamma, ln_beta, v, g, bias, widths):
    karr = np.arange(MAX_BW)
    bw_mask = karr[None, :] < widths[:, None]
    fm = (
        np.broadcast_to(bw_mask[:, None, :, None], (NB, C, MAX_BW, 2))
        .reshape(NB, D)
        .astype(np.float32)
    )
    vnorm = np.sqrt((v * v).sum(-1, keepdims=True))
    W = g[..., None] * v / vnorm
    W2 = W * (ln_gamma * fm)[:, None, :]
    bias2 = np.einsum("ned,nd->ne", W, ln_beta * fm) + bias
    W2p = W2[:, :, _feature_perm()]  # [NB, E, D] in (k,c,r) order
    return W2p, bias2


def _pack_weights(W2p, widths):
    """Pack per-band [k_n = 4w, E] weight rows into SBUF-layout chunks of 128."""
    kns = np.maximum(4 * widths, 4).astype(np.int64)
    nchunks = np.maximum(1, (kns + 127) // 128).astype(np.int64)
    tot_chunks = int(nchunks.sum())
    Wt = np.zeros((128, tot_chunks * 128), np.float32)
    chunk_base = np.zeros(NB, np.int64)
    cb = 0
    for n in range(NB):
        chunk_base[n] = cb
        kn = int(kns[n])
        w4 = 4 * int(widths[n])
        col = np.zeros((kn, E), np.float32)
        if w4 > 0:
            col[:w4] = W2p[n, :, :w4].T
        for j in range(int(nchunks[n])):
            cs = min(128, kn - j * 128)
            Wt[:cs, (cb + j) * 128 : (cb + j) * 128 + E] = col[j * 128 : j * 128 + cs]
        cb += int(nchunks[n])
    return Wt, kns, nchunks, chunk_base, tot_chunks


def _prep_x(x):
    """x [B,C,F,T,2] f32 -> x4 [NCORES, B, TLOC, FPAD*4] with (k,c,r) features, padded."""
    xr = np.transpose(x, (0, 3, 2, 1, 4)).reshape(B, T, F, 4)  # [B,T,F,(c,r)]
    x4 = np.empty((B, T, FPAD, 4), np.float32)
    x4[:, :, :F, :] = xr
    x4[:, :, F:, :] = xr[:, :, F - 1 : F, :]
    x4 = x4.reshape(B, NCORES, TLOC, FPAD * 4)
    x4 = np.ascontiguousarray(np.transpose(x4, (1, 0, 2, 3)))
    return x4  # [NCORES, B, TLOC, FPAD*4]


def _build_program(kns, nchunks, chunk_base, tot_chunks, starts):
    import concourse.bass as bass
    import concourse.bacc as bacc
    import concourse.tile as tile
    from concourse import mybir
    from concourse.masks import make_identity
    from contextlib import ExitStack

    f32 = mybir.dt.float32
    bf16 = mybir.dt.bfloat16
    nc = bacc.Bacc()
    x_ext = nc.declare_dram_parameter("x4", [B, TLOC, FPAD * 4], bf16, isOutput=False)
    wt_ext = nc.declare_dram_parameter(
        "wt", [128, tot_chunks * 128], bf16, isOutput=False
    )
    bias_ext = nc.declare_dram_parameter("bias2", [E, NB], f32, isOutput=False)
    z_ext = nc.declare_dram_parameter("out", [NB, B, E, TLOC], bf16, isOutput=True)

    with ExitStack() as ctx:
        tc = ctx.enter_context(tile.TileContext(nc))
        consts = ctx.enter_context(tc.tile_pool(name="consts", bufs=1))
        slabs = ctx.enter_context(tc.tile_pool(name="slabs", bufs=1))
        stats = ctx.enter_context(tc.tile_pool(name="stats", bufs=8))
        xh_pool = ctx.enter_context(tc.tile_pool(name="xh", bufs=4))
        xt_pool = ctx.enter_context(tc.tile_pool(name="xt", bufs=6))
        zs_pool = ctx.enter_context(tc.tile_pool(name="zs", bufs=4))
        tp_psum = ctx.enter_context(tc.tile_pool(name="tp", bufs=4, space="PSUM"))
        z_psum = ctx.enter_context(tc.tile_pool(name="zp", bufs=3, space="PSUM"))

        ident = consts.tile([128, 128], bf16)
        make_identity(nc, ident)
        eps_t = consts.tile([128, 1], f32)
        nc.vector.memset(eps_t, EPS)
        wt_sb = consts.tile([128, tot_chunks * 128], bf16)
        nc.sync.dma_start(out=wt_sb, in_=wt_ext[:, :])
        bias_sb = consts.tile([E, NB], f32)
        nc.sync.dma_start(out=bias_sb, in_=bias_ext[:, :])

        slab_tiles = {}
        for b in range(B):
            for t0 in range(TLOC // 128):
                st = slabs.tile([128, FPAD * 4], bf16, tag=f"slab_{b}_{t0}")
                nc.sync.dma_start(
                    out=st, in_=x_ext[b, t0 * 128 : (t0 + 1) * 128, :]
                )
                slab_tiles[(b, t0)] = st

        NSUB = B * (TLOC // 128)  # 8 stat subtiles per band
        for n in range(NB):
            kn = int(kns[n])
            s4 = 4 * int(starts[n])
            nch = int(nchunks[n])
            cb = int(chunk_base[n])
            # --- stats for all 8 subtiles of this band, batched scalars ---
            mvb = stats.tile([128, 2 * NSUB], f32, tag="mvb")
            for i, (b, t0) in enumerate(
                (b, t0) for b in range(B) for t0 in range(TLOC // 128)
            ):
                xsl = slab_tiles[(b, t0)][:, s4 : s4 + kn]
                stt = stats.tile([128, 6], f32)
                nc.vector.bn_stats(out=stt, in_=xsl)
                nc.vector.bn_aggr(out=mvb[:, 2 * i : 2 * i + 2], in_=stt)
            vrb = stats.tile([128, NSUB], f32, tag="vrb")
            nc.vector.tensor_copy(out=vrb, in_=mvb[:, 1 : 2 * NSUB : 2])
            rsb = stats.tile([128, NSUB], f32, tag="rsb")
            nc.scalar.activation(
                out=rsb,
                in_=vrb,
                func=mybir.ActivationFunctionType.Sqrt,
                bias=eps_t,
                scale=1.0,
            )
            nc.vector.reciprocal(out=rsb, in_=rsb)
            mvh = mvb
            rsh = rsb
            for b in range(B):
                xhs = []
                for t0 in range(TLOC // 128):
                    i = b * (TLOC // 128) + t0
                    xsl = slab_tiles[(b, t0)][:, s4 : s4 + kn]
                    xh_t = xh_pool.tile([128, 260], bf16)
                    nc.vector.tensor_scalar(
                        out=xh_t[:, :kn],
                        in0=xsl,
                        scalar1=mvh[:, 2 * i : 2 * i + 1],
                        scalar2=rsh[:, i : i + 1],
                        op0=mybir.AluOpType.subtract,
                        op1=mybir.AluOpType.mult,
                    )
                    xhs.append(xh_t)
                zp = z_psum.tile([128, 256], f32)
                for j in range(nch):
                    cs = min(128, kn - j * 128)
                    tp = tp_psum.tile([128, 256], bf16)
                    for t0 in range(2):
                        nc.tensor.transpose(
                            out=tp[:cs, t0 * 128 : (t0 + 1) * 128],
                            in_=xhs[t0][:, j * 128 : j * 128 + cs],
                            identity=ident,
                        )
                    xt = xt_pool.tile([128, 256], bf16)
                    nc.vector.tensor_copy(out=xt[:cs, :], in_=tp[:cs, :])
                    nc.tensor.matmul(
                        zp,
                        lhsT=wt_sb[:cs, (cb + j) * 128 : (cb + j) * 128 + E],
                        rhs=xt[:cs, :],
                        start=(j == 0),
                        stop=(j == nch - 1),
                    )
                zs = zs_pool.tile([128, 256], bf16)
                nc.scalar.activation(
                    out=zs,
                    in_=zp,
                    func=mybir.ActivationFunctionType.Identity,
                    bias=bias_sb[:, n : n + 1],
                    scale=1.0,
                )
                nc.sync.dma_start(out=z_ext[n, b, :, :], in_=zs)
    nc.compile()
    return nc


def kernel(x, ln_gamma, ln_beta, v, g, bias, band_start, band_width):
    global LAST_EXEC_NS
    _ensure_trace_hook()
    from concourse.bass_utils import run_bass_kernel_spmd

    x = np.asarray(x, np.float32)
    ln_gamma = np.asarray(ln_gamma, np.float32)
    ln_beta = np.asarray(ln_beta, np.float32)
    v = np.asarray(v, np.float32)
    g = np.asarray(g, np.float32)
    bias = np.asarray(bias, np.float32)
    starts = np.asarray(band_start).astype(np.int64)
    widths = np.asarray(band_width).astype(np.int64)

    import ml_dtypes

    W2p, bias2 = _fold_weights(ln_gamma, ln_beta, v, g, bias, widths)
    Wt, kns, nchunks, chunk_base, tot_chunks = _pack_weights(W2p, widths)
    x4 = _prep_x(x)

    bf = ml_dtypes.bfloat16
    x4b = x4.astype(bf)
    Wtb = Wt.astype(bf)
    bias2t = np.ascontiguousarray(bias2.T)  # [E, NB] f32

    key = (tuple(starts.tolist()), tuple(widths.tolist()))
    if key not in _PLAN_CACHE:
        _PLAN_CACHE[key] = _build_program(
            kns, nchunks, chunk_base, tot_chunks, starts
        )
    nc = _PLAN_CACHE[key]

    in_maps = [
        {"x4": x4b[i], "wt": Wtb, "bias2": bias2t} for i in range(NCORES)
    ]
    res = run_bass_kernel_spmd(nc, in_maps, core_ids=list(range(NCORES)))
    LAST_EXEC_NS = res.exec_time_ns

    zarr = np.stack([np.asarray(r["out"]) for r in res.results]).astype(
        np.float32
    )  # [8, NB, B, E, TLOC]
    z = np.transpose(zarr, (2, 1, 0, 4, 3)).reshape(B, NB, T, E)
    return np.ascontiguousarray(z)


# revision 18
# speedup vs baseline: 2.4088x; 1.1604x over previous
"""Trainium2 Bass kernel for BandSplitModule (masked LN per band + weight-normed Linear).

Strategy:
  - Data-parallel over T (2048 = 8 cores x 256). No collectives.
  - Host folds weight-norm + LN affine into a single per-band weight matrix
    W2[n] = (g * v / ||v||) * (gamma * mask) with a bias row
    bias2[n] = W @ (beta * mask) + bias, prepended as contraction row 0
    (the device appends a ones column to xhat so the matmul adds the bias).
  - Features permuted from (c, k, reim) to (k, c, reim) order so each band's
    features are one contiguous slice of a [t=128, (F+64)*4] SBUF slab.
  - Runtime band_start/band_width are baked into the compiled program
    (compilation happens inside kernel(); results cached per band structure).
  - Device per band-tile: bn_stats/bn_aggr -> rsqrt -> tensor_scalar xhat,
    TensorE transpose -> matmul (k = 4w+1 chunks) -> z[E, T] psum -> out.
"""
import os
import numpy as np

B, C, F, T, E = 4, 2, 1025, 2048, 128
MAX_BW = 65
NB = 37
EPS = 1e-5
NCORES = 8
TLOC = T // NCORES  # 256
FPAD = F + MAX_BW - 1  # 1089
D = C * MAX_BW * 2  # 260

LAST_EXEC_NS = None

_PLAN_CACHE = {}


def _ensure_trace_hook():
    """Install the antenv.axon_hooks NTFF-profile shim (missing on this image)
    so run_bass_kernel_spmd(trace=True) can capture HW exec time. Fully
    optional — any failure leaves the plain execution path untouched."""
    try:
        import sys, types

        if "antenv.axon_hooks" not in sys.modules:
            mod = types.ModuleType("antenv.axon_hooks")
            _h = {"hook": None}
            mod.set_axon_ntff_profile_hook = lambda h: _h.__setitem__("hook", h)
            mod.get_axon_ntff_profile_hook = lambda: _h["hook"]
            sys.modules["antenv.axon_hooks"] = mod
            try:
                import antenv

                antenv.axon_hooks = mod
            except Exception:
                pass
            try:
                from trn_agent_boot.trn_boot import _ntff_profile_via_ctypes

                hook = _ntff_profile_via_ctypes("/opt/axon/libaxon_pjrt.so")
                if hook is not None:
                    mod.set_axon_ntff_profile_hook(hook)
            except Exception:
                pass
        import concourse.bass_utils as bu

        if not getattr(bu, "_offline_upload_patch", False):
            bu.upload_artifacts = lambda tmpdir: tmpdir
            bu._offline_upload_patch = True
    except Exception:
        pass


def _feature_perm():
    # new index (k,c,r) -> reference index (c,k,r)
    kk, cc, rr = np.meshgrid(
        np.arange(MAX_BW), np.arange(C), np.arange(2), indexing="ij"
    )
    new_i = (kk * 4 + cc * 2 + rr).reshape(-1)
    src_i = (cc * (MAX_BW * 2) + kk * 2 + rr).reshape(-1)
    perm = np.empty(D, np.int64)
    perm[new_i] = src_i
    return perm


def _fold_weights(ln_gamma, ln_beta, v, g, bias, widths):
    karr = np.arange(MAX_BW)
    bw_mask = karr[None, :] < widths[:, None]
    fm = (
        np.broadcast_to(bw_mask[:, None, :, None], (NB, C, MAX_BW, 2))
        .reshape(NB, D)
        .astype(np.float32)
    )
    vnorm = np.sqrt((v * v).sum(-1, keepdims=True))
    W = g[..., None] * v / vnorm
    W2 = W * (ln_gamma * fm)[:, None, :]
    bias2 = np.einsum("ned,nd->ne", W, ln_beta * fm) + bias
    W2p = W2[:, :, _feature_perm()]  # [NB, E, D] in (k,c,r) order
    return W2p, bias2


def _pack_weights(W2p, widths):
    """Pack per-band [k_n = 4w, E] weight rows into SBUF-layout chunks of 128."""
    kns = np.maximum(4 * widths, 4).astype(np.int64)
    nchunks = np.maximum(1, (kns + 127) // 128).astype(np.int64)
    tot_chunks = int(nchunks.sum())
    Wt = np.zeros((128, tot_chunks * 128), np.float32)
    chunk_base = np.zeros(NB, np.int64)
    cb = 0
    for n in range(NB):
        chunk_base[n] = cb
        kn = int(kns[n])
        w4 = 4 * int(widths[n])
        col = np.zeros((kn, E), np.float32)
        if w4 > 0:
            col[:w4] = W2p[n, :, :w4].T
        for j in range(int(nchunks[n])):
            cs = min(128, kn - j * 128)
            Wt[:cs, (cb + j) * 128 : (cb + j) * 128 + E] = col[j * 128 : j * 128 + cs]
        cb += int(nchunks[n])
    return Wt, kns, nchunks, chunk_base, tot_chunks


def _prep_x(x):
    """x [B,C,F,T,2] f32 -> x4 [NCORES, B, TLOC, FPAD*4] with (k,c,r) features, padded."""
    xr = np.transpose(x, (0, 3, 2, 1, 4)).reshape(B, T, F, 4)  # [B,T,F,(c,r)]
    x4 = np.empty((B, T, FPAD, 4), np.float32)
    x4[:, :, :F, :] = xr
    x4[:, :, F:, :] = xr[:, :, F - 1 : F, :]
    x4 = x4.reshape(B, NCORES, TLOC, FPAD * 4)
    x4 = np.ascontiguousarray(np.transpose(x4, (1, 0, 2, 3)))
    return x4  # [NCORES, B, TLOC, FPAD*4]


def _build_program(kns, nchunks, chunk_base, tot_chunks, starts):
    import concourse.bass as bass
    import concourse.bacc as bacc
    import concourse.tile as tile
    from concourse import mybir
    from concourse.masks import make_identity
    from contextlib import ExitStack

    f32 = mybir.dt.float32
    bf16 = mybir.dt.bfloat16
    nc = bacc.Bacc()
    x_ext = nc.declare_dram_parameter("x4", [B, TLOC, FPAD * 4], bf16, isOutput=False)
    wt_ext = nc.declare_dram_parameter(
        "wt", [128, tot_chunks * 128], bf16, isOutput=False
    )
    bias_ext = nc.declare_dram_parameter("bias2", [E, NB], f32, isOutput=False)
    z_ext = nc.declare_dram_parameter("out", [NB, B, E, TLOC], bf16, isOutput=True)

    with ExitStack() as ctx:
        tc = ctx.enter_context(tile.TileContext(nc))
        consts = ctx.enter_context(tc.tile_pool(name="consts", bufs=1))
        slabs = ctx.enter_context(tc.tile_pool(name="slabs", bufs=1))
        stats = ctx.enter_context(tc.tile_pool(name="stats", bufs=8))
        xh_pool = ctx.enter_context(tc.tile_pool(name="xh", bufs=4))
        xt_pool = ctx.enter_context(tc.tile_pool(name="xt", bufs=6))
        zs_pool = ctx.enter_context(tc.tile_pool(name="zs", bufs=4))
        tp_psum = ctx.enter_context(tc.tile_pool(name="tp", bufs=4, space="PSUM"))
        z_psum = ctx.enter_context(tc.tile_pool(name="zp", bufs=3, space="PSUM"))

        ident = consts.tile([128, 128], bf16)
        make_identity(nc, ident)
        eps_t = consts.tile([128, 1], f32)
        nc.vector.memset(eps_t, EPS)
        wt_sb = consts.tile([128, tot_chunks * 128], bf16)
        nc.sync.dma_start(out=wt_sb, in_=wt_ext[:, :])
        bias_sb = consts.tile([E, NB], f32)
        nc.sync.dma_start(out=bias_sb, in_=bias_ext[:, :])

        slab_tiles = {}
        for b in range(B):
            for t0 in range(TLOC // 128):
                st = slabs.tile([128, FPAD * 4], bf16, tag=f"slab_{b}_{t0}")
                nc.sync.dma_start(
                    out=st, in_=x_ext[b, t0 * 128 : (t0 + 1) * 128, :]
                )
                slab_tiles[(b, t0)] = st

        NSUB = B * (TLOC // 128)  # 8 stat subtiles per band
        for n in range(NB):
            kn = int(kns[n])
            s4 = 4 * int(starts[n])
            nch = int(nchunks[n])
            cb = int(chunk_base[n])
            # --- stats for all 8 subtiles of this band, batched scalars ---
            mvb = stats.tile([128, 2 * NSUB], f32, tag="mvb")
            for i, (b, t0) in enumerate(
                (b, t0) for b in range(B) for t0 in range(TLOC // 128)
            ):
                xsl = slab_tiles[(b, t0)][:, s4 : s4 + kn]
                stt = stats.tile([128, 6], f32)
                nc.vector.bn_stats(out=stt, in_=xsl)
                nc.vector.bn_aggr(out=mvb[:, 2 * i : 2 * i + 2], in_=stt)
            vrb = stats.tile([128, NSUB], f32, tag="vrb")
            nc.vector.tensor_copy(out=vrb, in_=mvb[:, 1 : 2 * NSUB : 2])
            rsb = stats.tile([128, NSUB], f32, tag="rsb")
            nc.scalar.activation(
                out=rsb,
                in_=vrb,
                func=mybir.ActivationFunctionType.Sqrt,
                bias=eps_t,
                scale=1.0,
            )
            nc.vector.reciprocal(out=rsb, in_=rsb)
            mrb = stats.tile([128, NSUB], f32, tag="mrb")
            nc.vector.tensor_mul(mrb, mvb[:, 0 : 2 * NSUB : 2], rsb)
            negmr = stats.tile([128, NSUB], f32, tag="negmr")
            nc.scalar.mul(out=negmr, in_=mrb, mul=-1.0)
            mvh = mvb
            rsh = rsb
            for b in range(B):
                xhs = []
                for t0 in range(TLOC // 128):
                    i = b * (TLOC // 128) + t0
                    xsl = slab_tiles[(b, t0)][:, s4 : s4 + kn]
                    xh_t = xh_pool.tile([128, 260], bf16)
                    if i % 2 == 0:
                        nc.vector.tensor_scalar(
                            out=xh_t[:, :kn],
                            in0=xsl,
                            scalar1=mvh[:, 2 * i : 2 * i + 1],
                            scalar2=rsh[:, i : i + 1],
                            op0=mybir.AluOpType.subtract,
                            op1=mybir.AluOpType.mult,
                        )
                    else:
                        nc.scalar.activation(
                            out=xh_t[:, :kn],
                            in_=xsl,
                            func=mybir.ActivationFunctionType.Identity,
                            scale=rsh[:, i : i + 1],
                            bias=negmr[:, i : i + 1],
                        )
                    xhs.append(xh_t)
                zp = z_psum.tile([128, 256], f32)
                for j in range(nch):
                    cs = min(128, kn - j * 128)
                    tp = tp_psum.tile([128, 256], bf16)
                    for t0 in range(2):
                        nc.tensor.transpose(
                            out=tp[:cs, t0 * 128 : (t0 + 1) * 128],
                            in_=xhs[t0][:, j * 128 : j * 128 + cs],
                            identity=ident,
                        )
                    xt = xt_pool.tile([128, 256], bf16)
                    nc.any.tensor_copy(out=xt[:cs, :], in_=tp[:cs, :])
                    nc.tensor.matmul(
                        zp,
                        lhsT=wt_sb[:cs, (cb + j) * 128 : (cb + j) * 128 + E],
                        rhs=xt[:cs, :],
                        start=(j == 0),
                        stop=(j == nch - 1),
                    )
                zs = zs_pool.tile([128, 256], bf16)
                nc.scalar.activation(
                    out=zs,
                    in_=zp,
                    func=mybir.ActivationFunctionType.Identity,
                    bias=bias_sb[:, n : n + 1],
                    scale=1.0,
                )
                nc.sync.dma_start(out=z_ext[n, b, :, :], in_=zs)
    nc.compile()
    return nc


def kernel(x, ln_gamma, ln_beta, v, g, bias, band_start, band_width):
    global LAST_EXEC_NS
    _ensure_trace_hook()
    from concourse.bass_utils import run_bass_kernel_spmd

    x = np.asarray(x, np.float32)
    ln_gamma = np.asarray(ln_gamma, np.float32)
    ln_beta = np.asarray(ln_beta, np.float32)
    v = np.asarray(v, np.float32)
    g = np.asarray(g, np.float32)
    bias = np.asarray(bias, np.float32)
    starts = np.asarray(band_start).astype(np.int64)
    widths = np.asarray(band_width).astype(np.int64)

    import ml_dtypes

    W2p, bias2 = _fold_weights(ln_gamma, ln_beta, v, g, bias, widths)
    Wt, kns, nchunks, chunk_base, tot_chunks = _pack_weights(W2p, widths)
    x4 = _prep_x(x)

    bf = ml_dtypes.bfloat16
    x4b = x4.astype(bf)
    Wtb = Wt.astype(bf)
    bias2t = np.ascontiguousarray(bias2.T)  # [E, NB] f32

    key = (tuple(starts.tolist()), tuple(widths.tolist()))
    if key not in _PLAN_CACHE:
        _PLAN_CACHE[key] = _build_program(
            kns, nchunks, chunk_base, tot_chunks, starts
        )
    nc = _PLAN_CACHE[key]

    in_maps = [
        {"x4": x4b[i], "wt": Wtb, "bias2": bias2t} for i in range(NCORES)
    ]
    res = run_bass_kernel_spmd(nc, in_maps, core_ids=list(range(NCORES)))
    LAST_EXEC_NS = res.exec_time_ns

    zarr = np.stack([np.asarray(r["out"]) for r in res.results]).astype(
        np.float32
    )  # [8, NB, B, E, TLOC]
    z = np.transpose(zarr, (2, 1, 0, 4, 3)).reshape(B, NB, T, E)
    return np.ascontiguousarray(z)


# revision 19
# speedup vs baseline: 2.4815x; 1.0302x over previous
"""Trainium2 Bass kernel for BandSplitModule (masked LN per band + weight-normed Linear).

Strategy:
  - Data-parallel over T (2048 = 8 cores x 256). No collectives.
  - Host folds weight-norm + LN affine into a single per-band weight matrix
    W2[n] = (g * v / ||v||) * (gamma * mask) with a bias row
    bias2[n] = W @ (beta * mask) + bias, prepended as contraction row 0
    (the device appends a ones column to xhat so the matmul adds the bias).
  - Features permuted from (c, k, reim) to (k, c, reim) order so each band's
    features are one contiguous slice of a [t=128, (F+64)*4] SBUF slab.
  - Runtime band_start/band_width are baked into the compiled program
    (compilation happens inside kernel(); results cached per band structure).
  - Device per band-tile: bn_stats/bn_aggr -> rsqrt -> tensor_scalar xhat,
    TensorE transpose -> matmul (k = 4w+1 chunks) -> z[E, T] psum -> out.
"""
import os
import numpy as np

B, C, F, T, E = 4, 2, 1025, 2048, 128
MAX_BW = 65
NB = 37
EPS = 1e-5
NCORES = 8
TLOC = T // NCORES  # 256
FPAD = F + MAX_BW - 1  # 1089
D = C * MAX_BW * 2  # 260

LAST_EXEC_NS = None

_PLAN_CACHE = {}


def _ensure_trace_hook():
    """Install the antenv.axon_hooks NTFF-profile shim (missing on this image)
    so run_bass_kernel_spmd(trace=True) can capture HW exec time. Fully
    optional — any failure leaves the plain execution path untouched."""
    try:
        import sys, types

        if "antenv.axon_hooks" not in sys.modules:
            mod = types.ModuleType("antenv.axon_hooks")
            _h = {"hook": None}
            mod.set_axon_ntff_profile_hook = lambda h: _h.__setitem__("hook", h)
            mod.get_axon_ntff_profile_hook = lambda: _h["hook"]
            sys.modules["antenv.axon_hooks"] = mod
            try:
                import antenv

                antenv.axon_hooks = mod
            except Exception:
                pass
            try:
                from trn_agent_boot.trn_boot import _ntff_profile_via_ctypes

                hook = _ntff_profile_via_ctypes("/opt/axon/libaxon_pjrt.so")
                if hook is not None:
                    mod.set_axon_ntff_profile_hook(hook)
            except Exception:
                pass
        import concourse.bass_utils as bu

        if not getattr(bu, "_offline_upload_patch", False):
            bu.upload_artifacts = lambda tmpdir: tmpdir
            bu._offline_upload_patch = True
    except Exception:
        pass


def _feature_perm():
    # new index (k,c,r) -> reference index (c,k,r)
    kk, cc, rr = np.meshgrid(
        np.arange(MAX_BW), np.arange(C), np.arange(2), indexing="ij"
    )
    new_i = (kk * 4 + cc * 2 + rr).reshape(-1)
    src_i = (cc * (MAX_BW * 2) + kk * 2 + rr).reshape(-1)
    perm = np.empty(D, np.int64)
    perm[new_i] = src_i
    return perm


def _fold_weights(ln_gamma, ln_beta, v, g, bias, widths):
    karr = np.arange(MAX_BW)
    bw_mask = karr[None, :] < widths[:, None]
    fm = (
        np.broadcast_to(bw_mask[:, None, :, None], (NB, C, MAX_BW, 2))
        .reshape(NB, D)
        .astype(np.float32)
    )
    vnorm = np.sqrt((v * v).sum(-1, keepdims=True))
    W = g[..., None] * v / vnorm
    W2 = W * (ln_gamma * fm)[:, None, :]
    bias2 = np.einsum("ned,nd->ne", W, ln_beta * fm) + bias
    W2p = W2[:, :, _feature_perm()]  # [NB, E, D] in (k,c,r) order
    return W2p, bias2


def _pack_weights(W2p, widths):
    """Pack per-band [k_n = 4w, E] weight rows into SBUF-layout chunks of 128."""
    kns = np.maximum(4 * widths, 4).astype(np.int64)
    nchunks = np.maximum(1, (kns + 127) // 128).astype(np.int64)
    tot_chunks = int(nchunks.sum())
    Wt = np.zeros((128, tot_chunks * 128), np.float32)
    chunk_base = np.zeros(NB, np.int64)
    cb = 0
    for n in range(NB):
        chunk_base[n] = cb
        kn = int(kns[n])
        w4 = 4 * int(widths[n])
        col = np.zeros((kn, E), np.float32)
        if w4 > 0:
            col[:w4] = W2p[n, :, :w4].T
        for j in range(int(nchunks[n])):
            cs = min(128, kn - j * 128)
            Wt[:cs, (cb + j) * 128 : (cb + j) * 128 + E] = col[j * 128 : j * 128 + cs]
        cb += int(nchunks[n])
    return Wt, kns, nchunks, chunk_base, tot_chunks


def _prep_x(x):
    """x [B,C,F,T,2] f32 -> x4 [NCORES, B, TLOC, FPAD*4] with (k,c,r) features, padded."""
    xr = np.transpose(x, (0, 3, 2, 1, 4)).reshape(B, T, F, 4)  # [B,T,F,(c,r)]
    x4 = np.empty((B, T, FPAD, 4), np.float32)
    x4[:, :, :F, :] = xr
    x4[:, :, F:, :] = xr[:, :, F - 1 : F, :]
    x4 = x4.reshape(B, NCORES, TLOC, FPAD * 4)
    x4 = np.ascontiguousarray(np.transpose(x4, (1, 0, 2, 3)))
    return x4  # [NCORES, B, TLOC, FPAD*4]


def _build_program(kns, nchunks, chunk_base, tot_chunks, starts):
    import concourse.bass as bass
    import concourse.bacc as bacc
    import concourse.tile as tile
    from concourse import mybir
    from concourse.masks import make_identity
    from contextlib import ExitStack

    f32 = mybir.dt.float32
    bf16 = mybir.dt.bfloat16
    nc = bacc.Bacc()
    x_ext = nc.declare_dram_parameter("x4", [B, TLOC, FPAD * 4], bf16, isOutput=False)
    wt_ext = nc.declare_dram_parameter(
        "wt", [128, tot_chunks * 128], bf16, isOutput=False
    )
    bias_ext = nc.declare_dram_parameter("bias2", [E, NB], f32, isOutput=False)
    z_ext = nc.declare_dram_parameter("out", [NB, B, E, TLOC], bf16, isOutput=True)

    with ExitStack() as ctx:
        tc = ctx.enter_context(tile.TileContext(nc))
        consts = ctx.enter_context(tc.tile_pool(name="consts", bufs=1))
        slabs = ctx.enter_context(tc.tile_pool(name="slabs", bufs=1))
        stats = ctx.enter_context(tc.tile_pool(name="stats", bufs=16))
        xh_pool = ctx.enter_context(tc.tile_pool(name="xh", bufs=8))
        xt_pool = ctx.enter_context(tc.tile_pool(name="xt", bufs=10))
        zs_pool = ctx.enter_context(tc.tile_pool(name="zs", bufs=6))
        tp_psum = ctx.enter_context(tc.tile_pool(name="tp", bufs=4, space="PSUM"))
        z_psum = ctx.enter_context(tc.tile_pool(name="zp", bufs=4, space="PSUM"))

        ident = consts.tile([128, 128], bf16)
        make_identity(nc, ident)
        eps_t = consts.tile([128, 1], f32)
        nc.vector.memset(eps_t, EPS)
        wt_sb = consts.tile([128, tot_chunks * 128], bf16)
        nc.sync.dma_start(out=wt_sb, in_=wt_ext[:, :])
        bias_sb = consts.tile([E, NB], f32)
        nc.sync.dma_start(out=bias_sb, in_=bias_ext[:, :])

        slab_tiles = {}
        for b in range(B):
            for t0 in range(TLOC // 128):
                st = slabs.tile([128, FPAD * 4], bf16, tag=f"slab_{b}_{t0}")
                nc.sync.dma_start(
                    out=st, in_=x_ext[b, t0 * 128 : (t0 + 1) * 128, :]
                )
                slab_tiles[(b, t0)] = st

        NSUB = B * (TLOC // 128)  # 8 stat subtiles per band
        for n in range(NB):
            kn = int(kns[n])
            s4 = 4 * int(starts[n])
            nch = int(nchunks[n])
            cb = int(chunk_base[n])
            # --- stats for all 8 subtiles of this band, batched scalars ---
            mvb = stats.tile([128, 2 * NSUB], f32, tag="mvb")
            for i, (b, t0) in enumerate(
                (b, t0) for b in range(B) for t0 in range(TLOC // 128)
            ):
                xsl = slab_tiles[(b, t0)][:, s4 : s4 + kn]
                stt = stats.tile([128, 6], f32)
                nc.vector.bn_stats(out=stt, in_=xsl)
                nc.vector.bn_aggr(out=mvb[:, 2 * i : 2 * i + 2], in_=stt)
            vrb = stats.tile([128, NSUB], f32, tag="vrb")
            nc.vector.tensor_copy(out=vrb, in_=mvb[:, 1 : 2 * NSUB : 2])
            rsb = stats.tile([128, NSUB], f32, tag="rsb")
            nc.scalar.activation(
                out=rsb,
                in_=vrb,
                func=mybir.ActivationFunctionType.Sqrt,
                bias=eps_t,
                scale=1.0,
            )
            nc.vector.reciprocal(out=rsb, in_=rsb)
            mrb = stats.tile([128, NSUB], f32, tag="mrb")
            nc.vector.tensor_mul(mrb, mvb[:, 0 : 2 * NSUB : 2], rsb)
            negmr = stats.tile([128, NSUB], f32, tag="negmr")
            nc.scalar.mul(out=negmr, in_=mrb, mul=-1.0)
            mvh = mvb
            rsh = rsb
            for b in range(B):
                xhs = []
                for t0 in range(TLOC // 128):
                    i = b * (TLOC // 128) + t0
                    xsl = slab_tiles[(b, t0)][:, s4 : s4 + kn]
                    xh_t = xh_pool.tile([128, 260], bf16)
                    if i % 2 == 0:
                        nc.vector.tensor_scalar(
                            out=xh_t[:, :kn],
                            in0=xsl,
                            scalar1=mvh[:, 2 * i : 2 * i + 1],
                            scalar2=rsh[:, i : i + 1],
                            op0=mybir.AluOpType.subtract,
                            op1=mybir.AluOpType.mult,
                        )
                    else:
                        nc.scalar.activation(
                            out=xh_t[:, :kn],
                            in_=xsl,
                            func=mybir.ActivationFunctionType.Identity,
                            scale=rsh[:, i : i + 1],
                            bias=negmr[:, i : i + 1],
                        )
                    xhs.append(xh_t)
                zp = z_psum.tile([128, 256], f32)
                for j in range(nch):
                    cs = min(128, kn - j * 128)
                    tp = tp_psum.tile([128, 256], bf16)
                    for t0 in range(2):
                        nc.tensor.transpose(
                            out=tp[:cs, t0 * 128 : (t0 + 1) * 128],
                            in_=xhs[t0][:, j * 128 : j * 128 + cs],
                            identity=ident,
                        )
                    xt = xt_pool.tile([128, 256], bf16)
                    nc.any.tensor_copy(out=xt[:cs, :], in_=tp[:cs, :])
                    nc.tensor.matmul(
                        zp,
                        lhsT=wt_sb[:cs, (cb + j) * 128 : (cb + j) * 128 + E],
                        rhs=xt[:cs, :],
                        start=(j == 0),
                        stop=(j == nch - 1),
                    )
                zs = zs_pool.tile([128, 256], bf16)
                nc.scalar.activation(
                    out=zs,
                    in_=zp,
                    func=mybir.ActivationFunctionType.Identity,
                    bias=bias_sb[:, n : n + 1],
                    scale=1.0,
                )
                nc.sync.dma_start(out=z_ext[n, b, :, :], in_=zs)
    nc.compile()
    return nc


def kernel(x, ln_gamma, ln_beta, v, g, bias, band_start, band_width):
    global LAST_EXEC_NS
    _ensure_trace_hook()
    from concourse.bass_utils import run_bass_kernel_spmd

    x = np.asarray(x, np.float32)
    ln_gamma = np.asarray(ln_gamma, np.float32)
    ln_beta = np.asarray(ln_beta, np.float32)
    v = np.asarray(v, np.float32)
    g = np.asarray(g, np.float32)
    bias = np.asarray(bias, np.float32)
    starts = np.asarray(band_start).astype(np.int64)
    widths = np.asarray(band_width).astype(np.int64)

    import ml_dtypes

    W2p, bias2 = _fold_weights(ln_gamma, ln_beta, v, g, bias, widths)
    Wt, kns, nchunks, chunk_base, tot_chunks = _pack_weights(W2p, widths)
    x4 = _prep_x(x)

    bf = ml_dtypes.bfloat16
    x4b = x4.astype(bf)
    Wtb = Wt.astype(bf)
    bias2t = np.ascontiguousarray(bias2.T)  # [E, NB] f32

    key = (tuple(starts.tolist()), tuple(widths.tolist()))
    if key not in _PLAN_CACHE:
        _PLAN_CACHE[key] = _build_program(
            kns, nchunks, chunk_base, tot_chunks, starts
        )
    nc = _PLAN_CACHE[key]

    in_maps = [
        {"x4": x4b[i], "wt": Wtb, "bias2": bias2t} for i in range(NCORES)
    ]
    res = run_bass_kernel_spmd(nc, in_maps, core_ids=list(range(NCORES)))
    LAST_EXEC_NS = res.exec_time_ns

    zarr = np.stack([np.asarray(r["out"]) for r in res.results]).astype(
        np.float32
    )  # [8, NB, B, E, TLOC]
    z = np.transpose(zarr, (2, 1, 0, 4, 3)).reshape(B, NB, T, E)
    return np.ascontiguousarray(z)
